# revision 91
# baseline (speedup 1.0000x reference)
"""Trainium2 Bass kernel for the S-LSTM (sentence-state LSTM) classifier.

Data-parallel over batch: 8 cores x 4 examples. Everything on-chip runs in a
"transposed" layout: feature channels on SBUF partitions, (example, position)
flattened on the free dim (4*128 = 512 columns). The per-step gate GEMM
computes gates.T = Wg.T @ ctx.T with Wg slices stationary and h.T moving at
N=512 (fp32 data in float32r mode -> full PE rate). Position shifts
(h_{i-1}, h_{i+1}, c shifts) are free-dim offsets into state tiles that carry
one zero guard column on each side of every example's 128 columns.

The global-node ("g") part of ctx is rank-1 along positions: gg = g @ Wg_g is
computed once per step as a tiny M=4 GEMM, then folded into the big GEMM as an
extra K chunk against a constant 0/1 selector matrix (zero-padded to K=128).

Weights are streamed from HBM each step in host-pre-tiled contiguous pieces
(one DMA per stationary block column), overlapped with PE work.
"""

import ml_dtypes
import numpy as np

import concourse.bass as bass
import concourse.mybir as mybir
from concourse import bacc
import concourse.tile as tile
from concourse.bass_utils import run_bass_kernel_spmd

F32 = mybir.dt.float32
F32R = mybir.dt.float32r
F16 = mybir.dt.float16
BF16 = mybir.dt.bfloat16
F8 = mybir.dt.float8e4
I32 = mybir.dt.int32
AL = mybir.AluOpType
AF = mybir.ActivationFunctionType
AX = mybir.AxisListType
DR = mybir.MatmulPerfMode.DoubleRow
WS = 64.0                 # fp8 weight pre-scale (PSUM holds WS*gates)
XS = 4.0                  # fp8 pre-scale for x (embeddings)

B, L, V, E, H, DOUT = 32, 128, 30000, 300, 512, 5
NUM_STEPS = 5
NCORES = 8
BL = B // NCORES          # 4 examples per core
N = BL * L                # 512 free columns
EP = 384                  # E padded to 3*128
HC = H // 128             # 4 H chunks
GC = 7 * H // 128         # 28 gate output chunks
KHH = 3 * HC              # 12 K chunks for hl/h/hr
EC = EP // 128            # 3 E chunks
GG_W = 7 * H + H          # 4096: [Wg_g | Wfi_g] columns
GGC = GG_W // 512         # 8
DP = 8                    # DOUT padded to even size for fp32r matmul


def build_nc():
    nc = bacc.Bacc(trn_type="TRN2", target_bir_lowering=False)

    d = {}

    def din(name, shape, dt=F32):
        d[name] = nc.dram_tensor(name, list(shape), dt, kind="ExternalInput")
        return d[name]

    # weights are host-pre-tiled so every DMA reads contiguous HBM
    embed_d = din("embed", (V, E))
    # DoubleRow fp8 pair tiles, all partition(p)-first so the bulk resident
    # loads iterate src/dst identically: [p, m, off, jj, i, c]; off 0..2 =
    # hl/h/hr blocks of WS*Wg, off 3 = (WS/XS)*Wg_x (zero-padded E->512)
    wg_all = din("wg_all", (128, GC, 4, 2, 2, 128), F8)
    wg_gcat = din("wg_gcat", (128, GGC, 2, 2, 512), F8)
    wfi_h = din("wfi_h", (128, 2, 2, H), F8)
    wfo_d = din("wfo", (128, HC, 2, 2 * HC, 128), F8)
    w0_d = din("w0", (128, HC, 2, 2, 128), F8)
    w1_d = din("w1", (128, 2 * HC, HC, 128), F8)
    w2_d = din("w2", (128, 2 * HC, DP), BF16)
    bg_t = din("bg_t", (128, GC))
    b0_t = din("b0_t", (128, HC))
    bfi_t = din("bfi_t", (128, HC))
    bgf_t = din("bgf_t", (128, HC))
    bgo_t = din("bgo_t", (128, HC))
    b1_t = din("b1_t", (128, 2 * HC))
    b2_r = din("b2_r", (BL, DP))
    sel_d = din("sel_m", (128, N), F32R)
    ident_d = din("ident", (128, 128))
    tok_d = din("tok_idx", (128, BL), I32)       # column e = tokens of example e
    mask_d = din("mask_rep", (128, N), BF16)
    invlen_d = din("invlen_rep", (128, BL), BF16)

    out_d = nc.dram_tensor("out", [BL, DOUT], F32, kind="ExternalOutput")

    with tile.TileContext(nc) as tc:
        with (
            tc.tile_pool(name="psumA", bufs=4, space="PSUM") as psumA,
            tc.tile_pool(name="psumB", bufs=2, space="PSUM") as psumB,
            tc.tile_pool(name="psumT", bufs=2, space="PSUM") as psumT,
            tc.tile_pool(name="gates", bufs=10) as p_gate,
            tc.tile_pool(name="tmp", bufs=12) as p_tmp,
            tc.tile_pool(name="small", bufs=28) as p_small,
            tc.tile_pool(name="state", bufs=1) as p_state,
        ):
            # ---------------- persistent state ----------------
            def T(shape, name, dt=F32):
                return p_state.tile(shape, dt, name=name, tag=name)

            # h state in fp8: three shifted copies so DoubleRow moving APs are
            # clean [128, 2, N] slices. hlT[l]=h[l-1], hrT[l]=h[l+1] (guard
            # columns per example stay zero from the prologue memset).
            hT = [T([128, HC, N], f"hT{i}", F8) for i in range(2)]
            hlT = [T([128, HC, N], f"hlT{i}", F8) for i in range(2)]
            hrT = [T([128, HC, N], f"hrT{i}", F8) for i in range(2)]
            # c state: central + shifted copies, all contiguous (strided
            # guarded views cost ~3x on DVE/GpSimd)
            cT = [T([128, HC, N], f"cT{i}", BF16) for i in range(2)]
            clT = [T([128, HC, N], f"clT{i}", BF16) for i in range(2)]
            crT = [T([128, HC, N], f"crT{i}", BF16) for i in range(2)]
            # g state fp8, free dim padded to 128: full-width DoubleRow
            # stationary tiles keep the 2-fp8/cycle fast path (out-partition
            # 16 ran at 427ns/MM vs 216ns full). Cols 4:128 stay zero.
            gT = [T([128, HC, 128], f"gT{i}", F8) for i in range(2)]
            cgT = [T([128, HC, BL], f"cgT{i}") for i in range(2)]
            xT = T([128, 4, N], "xT", F8)   # XS*x, chunk 3 zero
            # gg rows 0:BL hold g @ [Wg_g | Wfi_g]; rows BL:128 stay zero so the
            # selector matmul can contract over a full K=128.
            gg_sb = T([128, GG_W], "gg_sb", F32R)
            x_nat = T([128, BL, EP], "x_nat")
            idx_sb = T([128, BL], "idx_sb", I32)
            mask_sb = T([128, N], "mask_sb", BF16)
            invlen_sb = T([128, BL], "invlen_sb", BF16)
            sel_sb = T([128, N], "sel_sb", F32R)
            ident_sb = T([128, 128], "ident_sb")
            wfi_sb = T([128, 2, 2, H], "wfi_sb", F8)
            # all step weights SBUF-resident (loaded once in the prologue, in
            # first-use order): no per-step weight DMA, no pool-free
            # semaphores clogging the sync queue
            # one tile per 4-chunk slab: readers wait only on their own
            # slab's DMA, not the whole 5.5MB load
            wall_sb = [
                T([128, 4, 4, 2, 2, 128], f"wall_sb{q}", F8) for q in range(7)
            ]
            wcat_sb = T([128, GGC, 2, 2, 512], "wcat_sb", F8)
            wfo_sb = T([128, HC, 2, 2 * HC, 128], "wfo_sb", F8)
            w0_sb = T([128, HC, 2, 2, 128], "w0_sb", F8)
            w1_sb = T([128, 2 * HC, HC, 128], "w1_sb", F8)
            w2_sb = T([128, 2 * HC, DP], "w2_sb", BF16)
            a1T = T([128, 2 * HC, BL], "a1T", BF16)
            bg_sb = T([128, GC], "bg_sb")
            b0_sb = T([128, HC], "b0_sb")
            bfi_sb = T([128, HC], "bfi_sb")
            bgf_sb = T([128, HC], "bgf_sb")
            bgo_sb = T([128, HC], "bgo_sb")
            b1_sb = T([128, 2 * HC], "b1_sb")
            b2_sb = T([BL, DP], "b2_sb")

            def mask3():
                return mask_sb[:].rearrange("p (e l) -> p e l", l=L)

            def v3(t):
                return t[:].rearrange("p (e l) -> p e l", l=L)

            def h3(t, hk):
                return t[:, hk].rearrange("p (e l) -> p e l", l=L)

            def emit_shift_copies(buf, hk):
                # hl[l]=h[l-1], hr[l]=h[l+1]; guard columns stay 0 (memset
                # once). On the sync DMA queue, which is empty now that all
                # weights are SBUF-resident (a backlogged FIFO queue would
                # delay these critical-path copies by many us).
                nc.sync.dma_start(
                    h3(hlT[buf], hk)[:, :, 1:L], h3(hT[buf], hk)[:, :, 0 : L - 1]
                )
                nc.sync.dma_start(
                    h3(hrT[buf], hk)[:, :, 0 : L - 1], h3(hT[buf], hk)[:, :, 1:L]
                )

            def emit_c_shift_copies(buf, hk):
                nc.sync.dma_start(
                    h3(clT[buf], hk)[:, :, 1:L], h3(cT[buf], hk)[:, :, 0 : L - 1]
                )
                nc.sync.dma_start(
                    h3(crT[buf], hk)[:, :, 0 : L - 1], h3(cT[buf], hk)[:, :, 1:L]
                )

            def tmp2(name, dt=BF16):
                return p_tmp.tile([128, N], dt, name=name, tag="tmp")

            def tmp3(name, dt=BF16):
                return p_tmp.tile([128, BL, L], dt, name=name, tag="tmp")

            def sm(name):
                return p_small.tile([128, BL], F32, name=name, tag="sm")

            # ---------------- prologue: loads (first-use order) ----------------
            nc.sync.dma_start(idx_sb[:], tok_d.ap())
            nc.sync.dma_start(ident_sb[:], ident_d.ap())
            nc.sync.dma_start(w0_sb[:], w0_d.ap())
            nc.sync.dma_start(mask_sb[:], mask_d.ap())
            nc.sync.dma_start(invlen_sb[:], invlen_d.ap())
            nc.sync.dma_start(sel_sb[:], sel_d.ap())
            for t_sb, t_d in (
                (bg_sb, bg_t), (b0_sb, b0_t), (bfi_sb, bfi_t),
                (bgf_sb, bgf_t), (bgo_sb, bgo_t), (b1_sb, b1_t), (b2_sb, b2_r),
            ):
                nc.sync.dma_start(t_sb[:], t_d.ap())
            # gate-GEMM weight slabs in first-use order; wcat after the first
            # two slabs (gg starts later than the first gate chunks)
            for q in range(2):
                nc.sync.dma_start(
                    wall_sb[q][:], wg_all.ap()[:, 4 * q : 4 * (q + 1)]
                )
            nc.sync.dma_start(wcat_sb[:], wg_gcat.ap())
            for q in range(2, 7):
                nc.sync.dma_start(
                    wall_sb[q][:], wg_all.ap()[:, 4 * q : 4 * (q + 1)]
                )
            nc.sync.dma_start(wfo_sb[:], wfo_d.ap())
            nc.sync.dma_start(wfi_sb[:], wfi_h.ap())
            nc.sync.dma_start(w1_sb[:], w1_d.ap())
            nc.sync.dma_start(w2_sb[:], w2_d.ap())

            # zero state (guard columns included)
            for t in (*hT, *hlT, *hrT, *gT):
                nc.vector.memset(t[:], 0.0)
            for t in (*cT, *clT, *crT, *cgT):
                nc.vector.memset(t[:], 0.0)
            nc.vector.memset(x_nat[:, :, E:], 0.0)  # pad cols only: gather writes [:E]
            nc.vector.memset(xT[:, 3].bitcast(F32), 0.0)  # zero K-pad chunk
            nc.vector.memset(gg_sb[:].bitcast(F32), 0.0)
            # fi slot masking: sel row BL holds (1-mask); pairing it with a
            # large negative constant in gg row BL drives masked fi logits to
            # -inf inside the PSUM accumulation (saves the efi*mask pass).
            # Rows 0:BL are overwritten by the per-step gg copy and rows
            # BL+1:32 pair with all-zero sel rows, so a 32-row memset is safe.
            nc.vector.memset(
                gg_sb[0:32, 7 * H :].bitcast(F32), -WS * 1e4
            )

            # ---------------- prologue: embedding gather + transpose ----------------
            for e in range(BL):
                nc.gpsimd.indirect_dma_start(
                    out=x_nat[:, e, :E],
                    out_offset=None,
                    in_=embed_d.ap(),
                    in_offset=bass.IndirectOffsetOnAxis(ap=idx_sb[:, e : e + 1], axis=0),
                )
            for e in range(BL):
                for ec in range(EC):
                    # alternate PSUM pools and copy engines so the 12
                    # transpose+copy pairs pipeline instead of serializing on
                    # 2 PSUM bufs
                    k12 = e * EC + ec
                    pool = (psumA, psumB, psumT)[k12 % 3]
                    pst = pool.tile(
                        [128, 128], F32, name="pst",
                        tag=("pA", "pB", "pT")[k12 % 3],
                    )
                    nc.tensor.transpose(
                        pst[:], x_nat[:, e, ec * 128 : (ec + 1) * 128], ident_sb[:]
                    )
                    dst = xT[:, ec, e * L : (e + 1) * L]
                    if k12 % 2 == 0:
                        nc.scalar.activation(dst, pst[:], AF.Identity, scale=XS)
                    else:
                        nc.vector.tensor_scalar(
                            dst, pst[:], XS, None, AL.mult
                        )

                # h0 for this example's columns only: starts as soon as this
                # example's transposes land instead of after all 12
                for hk in range(HC):
                    ps0 = psumA.tile([128, L], F32, name="ps_h0", tag="pA")
                    for jj in range(2):
                        nc.tensor.matmul(
                            ps0[:],
                            w0_sb[:, hk, jj],
                            xT[:, 2 * jj : 2 * jj + 2, e * L : (e + 1) * L],
                            start=(jj == 0), stop=(jj == 1), perf_mode=DR,
                        )
                    h0t = p_small.tile([128, L], BF16, name="h0t", tag="h0t")
                    nc.scalar.activation(
                        h0t[:], ps0[:], AF.Tanh, bias=b0_sb[:, hk : hk + 1],
                        scale=1.0 / WS,
                    )
                    nc.vector.tensor_mul(
                        out=h3(hT[0], hk)[:, e], in0=h0t[:],
                        in1=mask3()[:, e],
                    )

            # ---------------- prologue: h0 shift copies + g0 ----------------
            for hk in range(HC):
                emit_shift_copies(0, hk)
                hsum = sm("hsum")
                nc.vector.reduce_sum(hsum[:], h3(hT[0], hk), axis=AX.X)
                nc.vector.tensor_mul(
                    out=gT[0][:, hk, 0:BL], in0=hsum[:], in1=invlen_sb[:]
                )

            # ---------------- steps ----------------
            for s in range(NUM_STEPS):
                cur, nxt = s % 2, (s + 1) % 2
                c_c, c_n = cT[cur], cT[nxt]
                g_c, g_n = gT[cur], gT[nxt]
                cg_c, cg_n = cgT[cur], cgT[nxt]

                def emit_gg(g_c=g_c):
                    # gg[0:BL] = g @ [Wg_g | Wfi_g]; column groups produced in
                    # the order the big-GEMM selectors consume them
                    for nj in (6, 0, 1, 2, 3, 4, 5, 7):
                        psg = psumB.tile([128, 512], F32, name="psg", tag="pB")
                        for kp in range(2):
                            nc.tensor.matmul(
                                psg[:], g_c[:, 2 * kp : 2 * kp + 2, :],
                                wcat_sb[:, nj, kp],
                                start=(kp == 0), stop=(kp == 1), perf_mode=DR,
                            )
                        nc.vector.tensor_copy(
                            out=gg_sb[0:BL, nj * 512 : (nj + 1) * 512],
                            in_=psg[0:BL],
                        )

                def emit_hmm(k_seq, cur=cur):
                    # 8 accumulating fp8 DoubleRow matmuls: hl/h/hr/x parts.
                    # wall_sb is host-permuted to use order, indexed by k_seq.
                    wp = wall_sb[k_seq // 4][:, k_seq % 4]
                    ps = psumA.tile([128, N], F32, name="ps_g", tag="pA")
                    # x block first (no dependence on running state), then the
                    # jj=0 pairs (h chunks 0,1) before jj=1 (chunks 2,3): the
                    # next step's GEMM can start before chunk-3 shift copies
                    # land
                    k = 0
                    for off, hx, jj in (
                        (3, xT, 0), (3, xT, 1),
                        (0, hlT[cur], 0), (1, hT[cur], 0), (2, hrT[cur], 0),
                        (0, hlT[cur], 1), (1, hT[cur], 1), (2, hrT[cur], 1),
                    ):
                        nc.tensor.matmul(
                            ps[:], wp[:, off, jj],
                            hx[:, 2 * jj : 2 * jj + 2, :],
                            start=(k == 0), stop=False, perf_mode=DR,
                        )
                        k += 1
                    return ps

                def emit_sel_evict(m, ps, j):
                    nc.tensor.matmul(
                        ps[:], gg_sb[:, m * 128 : (m + 1) * 128], sel_sb[:],
                        start=False, stop=True,
                    )
                    et = p_gate.tile([128, N], BF16, name=f"eg{j}", tag="gate")
                    fn = AF.Exp if j < 5 else (AF.Sigmoid if j == 5 else AF.Tanh)
                    nc.scalar.activation(
                        et[:], ps[:], fn, scale=1.0 / WS, bias=bg_sb[:, m : m + 1]
                    )
                    return et

                J_ORDER = (6, 0, 1, 2, 3, 4, 5)  # u first, exps, o last
                h_avg = []
                for hk in range(HC):
                    eg = {}
                    for idx, j in enumerate(J_ORDER):
                        m = j * HC + hk
                        ps = emit_hmm(hk * 7 + idx)
                        if hk == 0 and idx == 0:
                            # gg GEMM goes here: its g_n dependency then hides
                            # under chunk 0's independent matmuls.
                            emit_gg()
                        eg[j] = emit_sel_evict(m, ps, j)
                        # emit recurrence ops as soon as inputs exist; ops on
                        # the post-e4 critical chain stay on DVE (a GpSimd hop
                        # costs ~1.2us), slack ops go to GpSimd
                        if idx == 1:
                            m1 = tmp2("m1")
                            nc.vector.tensor_mul(m1[:], eg[0][:], eg[6][:])
                        elif idx == 2:
                            s01 = tmp2("s01")
                            nc.gpsimd.tensor_add(s01[:], eg[0][:], eg[1][:])
                            t1 = tmp2("t1")
                            nc.gpsimd.tensor_mul(t1[:], eg[1][:], clT[cur][:, hk])
                        elif idx == 3:
                            t2 = tmp2("t2")
                            nc.gpsimd.tensor_mul(t2[:], eg[2][:], cT[cur][:, hk])
                        elif idx == 4:
                            s23 = tmp2("s23")
                            nc.gpsimd.tensor_add(s23[:], eg[2][:], eg[3][:])
                            t3 = tmp2("t3")
                            nc.vector.tensor_mul(t3[:], eg[3][:], crT[cur][:, hk])
                            p12 = tmp2("p12")
                            nc.vector.tensor_add(p12[:], t1[:], t2[:])
                            s0123 = tmp2("s0123")
                            nc.vector.tensor_add(s0123[:], s01[:], s23[:])
                            a012 = tmp2("a012")
                            nc.vector.tensor_add(a012[:], p12[:], m1[:])
                            a0123 = tmp2("a0123")
                            nc.vector.tensor_add(a0123[:], a012[:], t3[:])
                        elif idx == 5:
                            S5 = tmp2("S5", F32)
                            nc.vector.tensor_add(S5[:], s0123[:], eg[4][:])
                            r5 = tmp2("r5", F32)
                            nc.vector.reciprocal_approx_fast(r5[:], S5[:])
                            rm = tmp2("rm")
                            nc.vector.tensor_mul(rm[:], r5[:], mask_sb[:])
                            # acc = a0123 + e4*cg, fused per example
                            acc = tmp2("acc")
                            for e in range(BL):
                                el = slice(e * L, (e + 1) * L)
                                nc.vector.scalar_tensor_tensor(
                                    out=acc[:, el], in0=eg[4][:, el],
                                    scalar=cg_c[:, hk, e : e + 1],
                                    in1=a0123[:, el],
                                    op0=AL.mult, op1=AL.add,
                                )
                            nc.vector.tensor_mul(
                                out=cT[nxt][:, hk], in0=acc[:], in1=rm[:]
                            )
                            tanh_c = tmp2("tanh_c")
                            nc.scalar.activation(
                                tanh_c[:], cT[nxt][:, hk], AF.Tanh
                            )
                            if s < NUM_STEPS - 1:
                                emit_c_shift_copies(nxt, hk)
                    # after the o gate: h_new and its average
                    nc.vector.tensor_mul(
                        out=hT[nxt][:, hk], in0=eg[5][:], in1=tanh_c[:]
                    )
                    if s < NUM_STEPS - 1:
                        # gate GEMM inputs for the next step only
                        emit_shift_copies(nxt, hk)
                    hsum = sm("hsum2")
                    nc.vector.reduce_sum(hsum[:], h3(hT[nxt], hk), axis=AX.X)
                    hav = p_small.tile([128, BL], F8, name="hav", tag="sm")
                    nc.vector.tensor_mul(hav[:], hsum[:], invlen_sb[:])
                    h_avg.append(hav)

                # -- fi GEMM: kc=3 (last-written h chunk) deferred to the end of
                # each accumulation so PE has ready work while h_n[3] finishes
                efims = []
                psfs = []
                for hk in range(HC):
                    psf = psumA.tile([128, N], F32, name="psf", tag="pA")
                    nc.tensor.matmul(
                        psf[:], wfi_sb[:, 0, :, hk * 128 : (hk + 1) * 128],
                        hT[nxt][:, 0:2, :],
                        start=True, stop=False, perf_mode=DR,
                    )
                    nc.tensor.matmul(
                        psf[:],
                        gg_sb[:, 7 * H + hk * 128 : 7 * H + (hk + 1) * 128],
                        sel_sb[:],
                        start=False, stop=False,
                    )
                    psfs.append(psf)
                # keep-warm punctuation: tiny throwaway matmuls that depend on
                # successive points of the hk=3 recurrence chain. They execute
                # spread across the tail wait, so the PE activity monitor never
                # sees a full idle window and the clock stays at 2.4 GHz.
                for dep in (rm[:, :128], acc[:, :128], tanh_c[:, :128]):
                    dmy = psumB.tile([64, 128], F32, name="dmy", tag="pB")
                    nc.tensor.matmul(
                        dmy[:, : dep.free_size()], mask_sb[:, :64], dep,
                        start=True, stop=True,
                    )
                for hk in range(HC):
                    psf = psfs[hk]
                    nc.tensor.matmul(
                        psf[:], wfi_sb[:, 1, :, hk * 128 : (hk + 1) * 128],
                        hT[nxt][:, 2:4, :],
                        start=False, stop=True, perf_mode=DR,
                    )
                    # masked slots already at -1e4 inside psf (sel row BL
                    # trick), so exp gives exact zeros: no mask pass needed
                    efi = tmp2("efi")
                    nc.scalar.activation(
                        efi[:], psf[:], AF.Exp, bias=bfi_sb[:, hk : hk + 1],
                        scale=1.0 / WS,
                    )
                    efims.append(efi)

                # -- fg / og GEMMs (transposed, N=4), in pairs with the
                # h_avg[3] contribution deferred to keep PE fed
                res_sm = {}
                for pair in range(HC):
                    mos = (2 * pair, 2 * pair + 1)
                    psts = []
                    wfp = wfo_sb[:, pair]
                    for wi, mo in enumerate(mos):
                        pst = psumT.tile([128, BL], F32, name="pst_f", tag="pT")
                        for kc in range(2 * HC - 1):
                            rhs = (
                                g_c[:, kc, 0:BL] if kc < HC
                                else h_avg[kc - HC][:]
                            )
                            nc.tensor.matmul(
                                pst[:], wfp[:, wi, kc], rhs,
                                start=(kc == 0), stop=False,
                            )
                        psts.append((pst, wi))
                    for (pst, wi), mo in zip(psts, mos):
                        mm = mo % HC
                        nc.tensor.matmul(
                            pst[:], wfp[:, wi, 2 * HC - 1], h_avg[HC - 1][:],
                            start=False, stop=True,
                        )
                        r_sm = sm("r_sm")
                        if mo < HC:
                            nc.scalar.activation(
                                r_sm[:], pst[:], AF.Exp,
                                bias=bgf_sb[:, mm : mm + 1], scale=1.0 / WS,
                            )
                        else:
                            nc.scalar.activation(
                                r_sm[:], pst[:], AF.Sigmoid,
                                bias=bgo_sb[:, mm : mm + 1], scale=1.0 / WS,
                            )
                        res_sm[mo] = r_sm
                efg = [res_sm[i] for i in range(HC)]
                ogs = [res_sm[HC + i] for i in range(HC)]

                # -- slot softmax + cg/g update (tcg tanh batched at the end:
                # one table load instead of four)
                for hk in range(HC):
                    efim = efims[hk]
                    pw = tmp2("pw")
                    nc.vector.tensor_mul(pw[:], efim[:], cT[nxt][:, hk])
                    s_c = sm("s_c")
                    nc.vector.reduce_sum(s_c[:], v3(pw), axis=AX.X)
                    ssum = sm("ssum")
                    nc.vector.reduce_sum(ssum[:], v3(efim), axis=AX.X)
                    den = sm("den")
                    nc.vector.tensor_add(den[:], efg[hk][:], ssum[:])
                    rden = sm("rden")
                    nc.vector.reciprocal(rden[:], den[:])
                    tnum = sm("tnum")
                    nc.vector.tensor_mul(tnum[:], efg[hk][:], cg_c[:, hk])
                    num = sm("num")
                    nc.vector.tensor_add(num[:], tnum[:], s_c[:])
                    nc.vector.tensor_mul(out=cg_n[:, hk], in0=num[:], in1=rden[:])
                for hk in range(HC):
                    tcg = sm("tcg")
                    nc.scalar.activation(tcg[:], cg_n[:, hk], AF.Tanh)
                    nc.vector.tensor_mul(
                        out=g_n[:, hk, 0:BL], in0=ogs[hk][:], in1=tcg[:]
                    )
                    # keep-warm through the slot-chain tail (f32 lhsT to match
                    # the f32 smalls)
                    dmy = psumB.tile([64, BL], F32, name="dmy2", tag="pB")
                    nc.tensor.matmul(
                        dmy[:], ident_sb[:, :64], tcg[:], start=True, stop=True,
                    )

            # ---------------- epilogue ----------------
            g_fin = gT[NUM_STEPS % 2]
            for pr in range(HC):
                # kc-major within a pair: the kc=0 matmuls start as soon as
                # g_fin chunk 0 lands instead of after the whole tcg loop
                psts = [
                    psumT.tile([128, BL], F32, name="pst_a1", tag="pT")
                    for _ in range(2)
                ]
                for kc in range(HC):
                    for wi in range(2):
                        nc.tensor.matmul(
                            psts[wi][:], w1_sb[:, 2 * pr + wi, kc],
                            g_fin[:, kc, 0:BL],
                            start=(kc == 0), stop=(kc == HC - 1),
                        )
                for wi in range(2):
                    mo = 2 * pr + wi
                    nc.scalar.activation(
                        a1T[:, mo], psts[wi][:], AF.Tanh,
                        bias=b1_sb[:, mo : mo + 1], scale=1.0 / WS,
                    )

            pslg = psumB.tile([BL, DP], F32, name="pslg", tag="pB")
            for kc in range(2 * HC):
                nc.tensor.matmul(
                    pslg[:], a1T[:, kc], w2_sb[:, kc],
                    start=(kc == 0), stop=(kc == 2 * HC - 1),
                )
            lg = p_small.tile([BL, DP], F32, name="lg", tag="lg")
            nc.vector.tensor_add(lg[:], pslg[:], b2_sb[:])
            mx = p_small.tile([BL, 1], F32, name="mx", tag="lg")
            nc.vector.reduce_max(mx[:], lg[:, :DOUT], axis=AX.X)
            tsh = p_small.tile([BL, DOUT], F32, name="tsh", tag="lg")
            nc.vector.tensor_scalar(tsh[:], lg[:, :DOUT], mx[:], None, AL.subtract)
            ex = p_small.tile([BL, DOUT], F32, name="ex", tag="lg")
            ssum = p_small.tile([BL, 1], F32, name="ssum_l", tag="lg")
            nc.scalar.activation(ex[:], tsh[:], AF.Exp, accum_out=ssum[:])
            lse = p_small.tile([BL, 1], F32, name="lse", tag="lg")
            nc.scalar.activation(lse[:], ssum[:], AF.Ln)
            res = p_small.tile([BL, DOUT], F32, name="res", tag="lg")
            nc.vector.tensor_scalar(res[:], tsh[:], lse[:], None, AL.subtract)
            nc.sync.dma_start(out_d.ap(), res[:])

    nc.compile()
    return nc


def prep_in_maps(inputs):
    """Host-side prep: slice per core, pad/retile weights. Returns in_maps."""
    tokens = np.asarray(inputs["tokens"]).astype(np.int32)
    lengths = np.asarray(inputs["lengths"]).astype(np.int32)
    f = lambda k: np.ascontiguousarray(np.asarray(inputs[k], dtype=np.float32))
    embed = f("embed")
    W0, b0 = f("W0"), f("b0")
    Wg, bg = f("Wg"), f("bg")
    Wgf, bgf = f("Wgf"), f("bgf")
    Wfi, bfi = f("Wfi"), f("bfi")
    Wgo, bgo = f("Wgo"), f("bgo")
    W1, b1 = f("W1"), f("b1")
    W2, b2 = f("W2"), f("b2")

    def tile_km(w, kc, mc):
        # [kc*128, mc*128] -> [mc, 128, kc, 128]: piece[m][p,k,c] = w[k*128+p, m*128+c]
        return np.ascontiguousarray(
            w.reshape(kc, 128, mc, 128).transpose(2, 1, 0, 3)
        )

    f8 = ml_dtypes.float8_e4m3fn
    WS, XS = 64.0, 4.0

    def tile_dr(w, scale, mc):
        # [kc*256, mc*128] -> [mc, 128, kc, 2, 128] DoubleRow fp8 pair tiles
        kc = w.shape[0] // 256
        return np.ascontiguousarray(
            (scale * w).reshape(kc, 2, 128, mc, 128).transpose(3, 2, 0, 1, 4)
        ).astype(f8)

    # [m, p, off, jj, i, c]: off 0..2 = hl/h/hr blocks, off 3 = x block
    wg_hhh = (
        (WS * Wg[: 3 * H]).reshape(3, 2, 2, 128, GC, 128).transpose(4, 3, 0, 1, 2, 5)
    )
    EPX = 512
    wg_x_pad = np.zeros((EPX, 7 * H), np.float32)
    wg_x_pad[:E] = (WS / XS) * Wg[3 * H : 3 * H + E]
    wg_xt = wg_x_pad.reshape(2, 2, 128, GC, 128).transpose(3, 2, 0, 1, 4)
    # permute the m axis into kernel emission order (hk-major, J_ORDER minor)
    # and move the partition axis first
    use_order = [j * HC + hk for hk in range(HC) for j in (6, 0, 1, 2, 3, 4, 5)]
    wg_all = np.ascontiguousarray(
        np.concatenate([wg_hhh, wg_xt[:, :, None]], axis=2)[use_order]
        .transpose(1, 0, 2, 3, 4, 5)
    ).astype(f8)
    gcat = WS * np.concatenate([Wg[3 * H + E :], Wfi[:H]], axis=1)  # [512, 4096]
    wg_gcat = np.ascontiguousarray(
        gcat.reshape(2, 2, 128, GGC, 512).transpose(2, 3, 0, 1, 4)
    ).astype(f8)
    wfi_hp = np.ascontiguousarray(
        (WS * Wfi[H:]).reshape(2, 2, 128, H).transpose(2, 0, 1, 3)
    ).astype(f8)
    wgf = tile_km(WS * Wgf, 2 * HC, HC)
    wgo = tile_km(WS * Wgo, 2 * HC, HC)
    wfo = np.ascontiguousarray(
        np.concatenate([wgf, wgo], axis=0)
        .reshape(HC, 2, 128, 2 * HC, 128).transpose(2, 0, 1, 3, 4)
    ).astype(f8)
    w0_pad = np.zeros((EPX, H), np.float32)
    w0_pad[:E] = W0
    w0 = np.ascontiguousarray(np.moveaxis(tile_dr(w0_pad, WS / XS, HC), 1, 0))
    w1 = np.ascontiguousarray(
        np.moveaxis(tile_km(WS * W1, HC, 2 * HC), 1, 0)
    ).astype(f8)
    w2p = np.zeros((2 * H, DP), np.float32)
    w2p[:, :DOUT] = W2
    b2p = np.zeros((DP,), np.float32)
    b2p[:DOUT] = b2
    w2 = np.ascontiguousarray(w2p.reshape(2 * HC, 128, DP).transpose(1, 0, 2))

    def t_bias(b):
        return np.ascontiguousarray(b.reshape(-1, 128).T)

    sel = np.zeros((128, N), np.float32)
    for e in range(BL):
        sel[e, e * L : (e + 1) * L] = 1.0
    ident = np.eye(128, dtype=np.float32)

    bf = ml_dtypes.bfloat16
    shared = dict(
        embed=embed, wg_all=wg_all,
        wg_gcat=wg_gcat, wfi_h=wfi_hp,
        wfo=wfo, w0=w0,
        w1=w1, w2=w2.astype(bf),
        bg_t=t_bias(bg), b0_t=t_bias(b0), bfi_t=t_bias(bfi), bgf_t=t_bias(bgf),
        bgo_t=t_bias(bgo), b1_t=t_bias(b1),
        b2_r=np.ascontiguousarray(np.tile(b2p[None, :], (BL, 1))),
        ident=ident,
    )

    in_maps = []
    for c in range(NCORES):
        sl = slice(c * BL, (c + 1) * BL)
        tok = tokens[sl]                                   # [BL, L]
        lens = np.maximum(lengths[sl].astype(np.float32), 1.0)
        mask = (np.arange(L)[None, :] < lengths[sl][:, None]).astype(np.float32)
        mask_rep = np.ascontiguousarray(
            np.broadcast_to(mask.reshape(1, N), (128, N)).astype(ml_dtypes.bfloat16)
        )
        invlen_rep = np.ascontiguousarray(
            np.broadcast_to((1.0 / lens).reshape(1, BL), (128, BL))
            .astype(ml_dtypes.bfloat16)
        )
        tok_idx = np.ascontiguousarray(tok.T.astype(np.int32))  # [L=128, BL]
        # sel row BL = (1-mask): pairs with gg row BL = -WS*1e4 to mask fi
        sel_m = sel.copy()
        sel_m[BL] = 1.0 - mask.reshape(N)
        m = dict(shared)
        m.update(tok_idx=tok_idx, mask_rep=mask_rep, invlen_rep=invlen_rep,
                 sel_m=sel_m)
        in_maps.append(m)
    return in_maps


_NC_CACHE = {}


def kernel(**inputs) -> np.ndarray:
    in_maps = prep_in_maps(inputs)
    if "nc" not in _NC_CACHE:
        _NC_CACHE["nc"] = build_nc()
    nc = _NC_CACHE["nc"]
    res = run_bass_kernel_spmd(nc, in_maps, core_ids=list(range(NCORES)))
    return np.concatenate([r["out"] for r in res.results], axis=0)


if __name__ == "__main__":
    nc = build_nc()
    print("built ok")



# revision 95
# speedup vs baseline: 1.0038x; 1.0038x over previous
"""Trainium2 Bass kernel for the S-LSTM (sentence-state LSTM) classifier.

Data-parallel over batch: 8 cores x 4 examples. Everything on-chip runs in a
"transposed" layout: feature channels on SBUF partitions, (example, position)
flattened on the free dim (4*128 = 512 columns). The per-step gate GEMM
computes gates.T = Wg.T @ ctx.T with Wg slices stationary and h.T moving at
N=512 (fp32 data in float32r mode -> full PE rate). Position shifts
(h_{i-1}, h_{i+1}, c shifts) are free-dim offsets into state tiles that carry
one zero guard column on each side of every example's 128 columns.

The global-node ("g") part of ctx is rank-1 along positions: gg = g @ Wg_g is
computed once per step as a tiny M=4 GEMM, then folded into the big GEMM as an
extra K chunk against a constant 0/1 selector matrix (zero-padded to K=128).

Weights are streamed from HBM each step in host-pre-tiled contiguous pieces
(one DMA per stationary block column), overlapped with PE work.
"""

import ml_dtypes
import numpy as np

import concourse.bass as bass
import concourse.mybir as mybir
from concourse import bacc
import concourse.tile as tile
from concourse.bass_utils import run_bass_kernel_spmd

F32 = mybir.dt.float32
F32R = mybir.dt.float32r
F16 = mybir.dt.float16
BF16 = mybir.dt.bfloat16
F8 = mybir.dt.float8e4
I32 = mybir.dt.int32
AL = mybir.AluOpType
AF = mybir.ActivationFunctionType
AX = mybir.AxisListType
DR = mybir.MatmulPerfMode.DoubleRow
WS = 64.0                 # fp8 weight pre-scale (PSUM holds WS*gates)
XS = 4.0                  # fp8 pre-scale for x (embeddings)

B, L, V, E, H, DOUT = 32, 128, 30000, 300, 512, 5
NUM_STEPS = 5
NCORES = 8
BL = B // NCORES          # 4 examples per core
N = BL * L                # 512 free columns
EP = 384                  # E padded to 3*128
HC = H // 128             # 4 H chunks
GC = 7 * H // 128         # 28 gate output chunks
KHH = 3 * HC              # 12 K chunks for hl/h/hr
EC = EP // 128            # 3 E chunks
GG_W = 7 * H + H          # 4096: [Wg_g | Wfi_g] columns
GGC = GG_W // 512         # 8
DP = 8                    # DOUT padded to even size for fp32r matmul


def build_nc():
    nc = bacc.Bacc(trn_type="TRN2", target_bir_lowering=False)

    d = {}

    def din(name, shape, dt=F32):
        d[name] = nc.dram_tensor(name, list(shape), dt, kind="ExternalInput")
        return d[name]

    # weights are host-pre-tiled so every DMA reads contiguous HBM
    embed_d = din("embed", (V, E))
    # DoubleRow fp8 pair tiles, all partition(p)-first so the bulk resident
    # loads iterate src/dst identically: [p, m, off, jj, i, c]; off 0..2 =
    # hl/h/hr blocks of WS*Wg, off 3 = (WS/XS)*Wg_x (zero-padded E->512)
    wg_all = din("wg_all", (128, GC, 4, 2, 2, 128), F8)
    wg_gcat = din("wg_gcat", (128, GGC, 2, 2, 512), F8)
    wfi_h = din("wfi_h", (128, 2, 2, H), F8)
    wfo_d = din("wfo", (128, HC, 2, 2 * HC, 128), F8)
    w0_d = din("w0", (128, HC, 2, 2, 128), F8)
    w1_d = din("w1", (128, 2 * HC, HC, 128), F8)
    w2_d = din("w2", (128, 2 * HC, DP), BF16)
    bg_t = din("bg_t", (128, GC))
    b0_t = din("b0_t", (128, HC))
    bfi_t = din("bfi_t", (128, HC))
    bgf_t = din("bgf_t", (128, HC))
    bgo_t = din("bgo_t", (128, HC))
    b1_t = din("b1_t", (128, 2 * HC))
    b2_r = din("b2_r", (BL, DP))
    sel_d = din("sel_m", (128, N), F32R)
    ident_d = din("ident", (128, 128))
    tok_d = din("tok_idx", (128, BL), I32)       # column e = tokens of example e
    mask_d = din("mask_rep", (128, N), BF16)
    invlen_d = din("invlen_rep", (128, BL), BF16)

    out_d = nc.dram_tensor("out", [BL, DOUT], F32, kind="ExternalOutput")

    with tile.TileContext(nc) as tc:
        with (
            tc.tile_pool(name="psumA", bufs=4, space="PSUM") as psumA,
            tc.tile_pool(name="psumB", bufs=2, space="PSUM") as psumB,
            tc.tile_pool(name="psumT", bufs=2, space="PSUM") as psumT,
            tc.tile_pool(name="gates", bufs=10) as p_gate,
            tc.tile_pool(name="tmp", bufs=12) as p_tmp,
            tc.tile_pool(name="small", bufs=28) as p_small,
            tc.tile_pool(name="state", bufs=1) as p_state,
        ):
            # ---------------- persistent state ----------------
            def T(shape, name, dt=F32):
                return p_state.tile(shape, dt, name=name, tag=name)

            # h state in fp8: three shifted copies so DoubleRow moving APs are
            # clean [128, 2, N] slices. hlT[l]=h[l-1], hrT[l]=h[l+1] (guard
            # columns per example stay zero from the prologue memset).
            hT = [T([128, HC, N], f"hT{i}", F8) for i in range(2)]
            hlT = [T([128, HC, N], f"hlT{i}", F8) for i in range(2)]
            hrT = [T([128, HC, N], f"hrT{i}", F8) for i in range(2)]
            # c state: central + shifted copies, all contiguous (strided
            # guarded views cost ~3x on DVE/GpSimd)
            cT = [T([128, HC, N], f"cT{i}", BF16) for i in range(2)]
            clT = [T([128, HC, N], f"clT{i}", BF16) for i in range(2)]
            crT = [T([128, HC, N], f"crT{i}", BF16) for i in range(2)]
            # g state fp8, free dim padded to 16 so DoubleRow lhsT pair slices
            # have 16B strides; cols 4:16 stay zero.
            gT = [T([128, HC, 16], f"gT{i}", F8) for i in range(2)]
            cgT = [T([128, HC, BL], f"cgT{i}") for i in range(2)]
            xT = T([128, 4, N], "xT", F8)   # XS*x, chunk 3 zero
            # gg rows 0:BL hold g @ [Wg_g | Wfi_g]; rows BL:128 stay zero so the
            # selector matmul can contract over a full K=128.
            gg_sb = T([128, GG_W], "gg_sb", F32R)
            x_nat = T([128, BL, EP], "x_nat")
            idx_sb = T([128, BL], "idx_sb", I32)
            mask_sb = T([128, N], "mask_sb", BF16)
            invlen_sb = T([128, BL], "invlen_sb", BF16)
            sel_sb = T([128, N], "sel_sb", F32R)
            ident_sb = T([128, 128], "ident_sb")
            wfi_sb = T([128, 2, 2, H], "wfi_sb", F8)
            # all step weights SBUF-resident (loaded once in the prologue, in
            # first-use order): no per-step weight DMA, no pool-free
            # semaphores clogging the sync queue
            # one tile per 4-chunk slab: readers wait only on their own
            # slab's DMA, not the whole 5.5MB load
            wall_sb = [
                T([128, 4, 4, 2, 2, 128], f"wall_sb{q}", F8) for q in range(7)
            ]
            wcat_sb = T([128, GGC, 2, 2, 512], "wcat_sb", F8)
            wfo_sb = T([128, HC, 2, 2 * HC, 128], "wfo_sb", F8)
            w0_sb = T([128, HC, 2, 2, 128], "w0_sb", F8)
            w1_sb = T([128, 2 * HC, HC, 128], "w1_sb", F8)
            w2_sb = T([128, 2 * HC, DP], "w2_sb", BF16)
            a1T = T([128, 2 * HC, BL], "a1T", BF16)
            bg_sb = T([128, GC], "bg_sb")
            b0_sb = T([128, HC], "b0_sb")
            bfi_sb = T([128, HC], "bfi_sb")
            bgf_sb = T([128, HC], "bgf_sb")
            bgo_sb = T([128, HC], "bgo_sb")
            b1_sb = T([128, 2 * HC], "b1_sb")
            b2_sb = T([BL, DP], "b2_sb")

            def mask3():
                return mask_sb[:].rearrange("p (e l) -> p e l", l=L)

            def v3(t):
                return t[:].rearrange("p (e l) -> p e l", l=L)

            def h3(t, hk):
                return t[:, hk].rearrange("p (e l) -> p e l", l=L)

            def emit_shift_copies(buf, hk):
                # hl[l]=h[l-1], hr[l]=h[l+1]; guard columns stay 0 (memset
                # once). On the sync DMA queue, which is empty now that all
                # weights are SBUF-resident (a backlogged FIFO queue would
                # delay these critical-path copies by many us).
                nc.sync.dma_start(
                    h3(hlT[buf], hk)[:, :, 1:L], h3(hT[buf], hk)[:, :, 0 : L - 1]
                )
                nc.sync.dma_start(
                    h3(hrT[buf], hk)[:, :, 0 : L - 1], h3(hT[buf], hk)[:, :, 1:L]
                )

            def emit_c_shift_copies(buf, hk):
                nc.sync.dma_start(
                    h3(clT[buf], hk)[:, :, 1:L], h3(cT[buf], hk)[:, :, 0 : L - 1]
                )
                nc.sync.dma_start(
                    h3(crT[buf], hk)[:, :, 0 : L - 1], h3(cT[buf], hk)[:, :, 1:L]
                )

            def tmp2(name, dt=BF16):
                return p_tmp.tile([128, N], dt, name=name, tag="tmp")

            def tmp3(name, dt=BF16):
                return p_tmp.tile([128, BL, L], dt, name=name, tag="tmp")

            def sm(name):
                return p_small.tile([128, BL], F32, name=name, tag="sm")

            # ---------------- prologue: loads (first-use order) ----------------
            nc.sync.dma_start(idx_sb[:], tok_d.ap())
            nc.sync.dma_start(ident_sb[:], ident_d.ap())
            nc.sync.dma_start(w0_sb[:], w0_d.ap())
            nc.sync.dma_start(mask_sb[:], mask_d.ap())
            nc.sync.dma_start(invlen_sb[:], invlen_d.ap())
            nc.sync.dma_start(sel_sb[:], sel_d.ap())
            for t_sb, t_d in (
                (bg_sb, bg_t), (b0_sb, b0_t), (bfi_sb, bfi_t),
                (bgf_sb, bgf_t), (bgo_sb, bgo_t), (b1_sb, b1_t), (b2_sb, b2_r),
            ):
                nc.sync.dma_start(t_sb[:], t_d.ap())
            # gate-GEMM weight slabs in first-use order; wcat after the first
            # two slabs (gg starts later than the first gate chunks)
            for q in range(2):
                nc.sync.dma_start(
                    wall_sb[q][:], wg_all.ap()[:, 4 * q : 4 * (q + 1)]
                )
            nc.sync.dma_start(wcat_sb[:], wg_gcat.ap())
            for q in range(2, 7):
                nc.sync.dma_start(
                    wall_sb[q][:], wg_all.ap()[:, 4 * q : 4 * (q + 1)]
                )
            nc.sync.dma_start(wfo_sb[:], wfo_d.ap())
            nc.sync.dma_start(wfi_sb[:], wfi_h.ap())
            nc.sync.dma_start(w1_sb[:], w1_d.ap())
            nc.sync.dma_start(w2_sb[:], w2_d.ap())

            # zero state (guard columns included)
            for t in (*hT, *hlT, *hrT, *gT):
                nc.vector.memset(t[:], 0.0)
            for t in (*cT, *clT, *crT, *cgT):
                nc.vector.memset(t[:], 0.0)
            nc.vector.memset(x_nat[:, :, E:], 0.0)  # pad cols only: gather writes [:E]
            nc.vector.memset(xT[:, 3].bitcast(F32), 0.0)  # zero K-pad chunk
            nc.vector.memset(gg_sb[:].bitcast(F32), 0.0)
            # fi slot masking: sel row BL holds (1-mask); pairing it with a
            # large negative constant in gg row BL drives masked fi logits to
            # -inf inside the PSUM accumulation (saves the efi*mask pass).
            # Rows 0:BL are overwritten by the per-step gg copy and rows
            # BL+1:32 pair with all-zero sel rows, so a 32-row memset is safe.
            nc.vector.memset(
                gg_sb[0:32, 7 * H :].bitcast(F32), -WS * 1e4
            )

            # ---------------- prologue: embedding gather + transpose ----------------
            for e in range(BL):
                nc.gpsimd.indirect_dma_start(
                    out=x_nat[:, e, :E],
                    out_offset=None,
                    in_=embed_d.ap(),
                    in_offset=bass.IndirectOffsetOnAxis(ap=idx_sb[:, e : e + 1], axis=0),
                )
            for e in range(BL):
                for ec in range(EC):
                    # alternate PSUM pools and copy engines so the 12
                    # transpose+copy pairs pipeline instead of serializing on
                    # 2 PSUM bufs
                    k12 = e * EC + ec
                    pool = (psumA, psumB, psumT)[k12 % 3]
                    pst = pool.tile(
                        [128, 128], F32, name="pst",
                        tag=("pA", "pB", "pT")[k12 % 3],
                    )
                    nc.tensor.transpose(
                        pst[:], x_nat[:, e, ec * 128 : (ec + 1) * 128], ident_sb[:]
                    )
                    dst = xT[:, ec, e * L : (e + 1) * L]
                    if k12 % 2 == 0:
                        nc.scalar.activation(dst, pst[:], AF.Identity, scale=XS)
                    else:
                        nc.vector.tensor_scalar(
                            dst, pst[:], XS, None, AL.mult
                        )

                # h0 for this example's columns only: starts as soon as this
                # example's transposes land instead of after all 12
                for hk in range(HC):
                    ps0 = psumA.tile([128, L], F32, name="ps_h0", tag="pA")
                    for jj in range(2):
                        nc.tensor.matmul(
                            ps0[:],
                            w0_sb[:, hk, jj],
                            xT[:, 2 * jj : 2 * jj + 2, e * L : (e + 1) * L],
                            start=(jj == 0), stop=(jj == 1), perf_mode=DR,
                        )
                    h0t = p_small.tile([128, L], BF16, name="h0t", tag="h0t")
                    nc.scalar.activation(
                        h0t[:], ps0[:], AF.Tanh, bias=b0_sb[:, hk : hk + 1],
                        scale=1.0 / WS,
                    )
                    nc.vector.tensor_mul(
                        out=h3(hT[0], hk)[:, e], in0=h0t[:],
                        in1=mask3()[:, e],
                    )

            # ---------------- prologue: h0 shift copies + g0 ----------------
            for hk in range(HC):
                emit_shift_copies(0, hk)
                hsum = sm("hsum")
                nc.vector.reduce_sum(hsum[:], h3(hT[0], hk), axis=AX.X)
                nc.vector.tensor_mul(
                    out=gT[0][:, hk, 0:BL], in0=hsum[:], in1=invlen_sb[:]
                )

            # ---------------- steps ----------------
            for s in range(NUM_STEPS):
                cur, nxt = s % 2, (s + 1) % 2
                c_c, c_n = cT[cur], cT[nxt]
                g_c, g_n = gT[cur], gT[nxt]
                cg_c, cg_n = cgT[cur], cgT[nxt]

                def emit_gg(g_c=g_c):
                    # gg[0:BL] = g @ [Wg_g | Wfi_g]; column groups produced in
                    # the order the big-GEMM selectors consume them
                    for nj in (6, 0, 1, 2, 3, 4, 5, 7):
                        psg = psumB.tile([16, 512], F32, name="psg", tag="pB")
                        for kp in range(2):
                            nc.tensor.matmul(
                                psg[:], g_c[:, 2 * kp : 2 * kp + 2, :],
                                wcat_sb[:, nj, kp],
                                start=(kp == 0), stop=(kp == 1), perf_mode=DR,
                            )
                        nc.vector.tensor_copy(
                            out=gg_sb[0:BL, nj * 512 : (nj + 1) * 512],
                            in_=psg[0:BL],
                        )

                # x block first (no dependence on running state), then the
                # jj=0 pairs (h chunks 0,1) before jj=1 (chunks 2,3): the
                # next step's GEMM can start before chunk-3 shift copies land
                PART0 = ((3, 0), (3, 1), (0, 0), (1, 0), (2, 0))
                PART1 = ((0, 1), (1, 1), (2, 1))

                def hx_of(off, cur=cur):
                    return (hlT[cur], hT[cur], hrT[cur], xT)[off]

                def emit_mms(ps, k_seq, sub, first):
                    wp = wall_sb[k_seq // 4][:, k_seq % 4]
                    for k, (off, jj) in enumerate(sub):
                        nc.tensor.matmul(
                            ps[:], wp[:, off, jj],
                            hx_of(off)[:, 2 * jj : 2 * jj + 2, :],
                            start=(first and k == 0), stop=False, perf_mode=DR,
                        )

                def emit_hmm(k_seq):
                    ps = psumA.tile([128, N], F32, name="ps_g", tag="pA")
                    emit_mms(ps, k_seq, PART0 + PART1, first=True)
                    return ps

                def emit_sel_evict(m, ps, j):
                    nc.tensor.matmul(
                        ps[:], gg_sb[:, m * 128 : (m + 1) * 128], sel_sb[:],
                        start=False, stop=True,
                    )
                    et = p_gate.tile([128, N], BF16, name=f"eg{j}", tag="gate")
                    fn = AF.Exp if j < 5 else (AF.Sigmoid if j == 5 else AF.Tanh)
                    nc.scalar.activation(
                        et[:], ps[:], fn, scale=1.0 / WS, bias=bg_sb[:, m : m + 1]
                    )
                    return et

                J_ORDER = (6, 0, 1, 2, 3, 4, 5)  # u first, exps, o last
                h_avg = []
                for hk in range(HC):
                    eg = {}
                    pending = {}
                    if hk == 0:
                        # pre-issue the copy-independent matmuls (x + chunk
                        # 0/1 pairs) of the first 4 groups: ~4us of PE runway
                        # while the previous step's chunk-2/3 shift copies and
                        # g_n resolve
                        for idx in range(4):
                            psp = psumA.tile([128, N], F32, name="ps_g", tag="pA")
                            emit_mms(psp, idx, PART0, first=True)
                            pending[idx] = psp
                    for idx, j in enumerate(J_ORDER):
                        m = j * HC + hk
                        if hk == 0 and idx in pending:
                            ps = pending.pop(idx)
                            emit_mms(ps, idx, PART1, first=False)
                        else:
                            ps = emit_hmm(hk * 7 + idx)
                        if hk == 0 and idx == 0:
                            # gg GEMM goes here: its g_n dependency then hides
                            # under chunk 0's independent matmuls.
                            emit_gg()
                        eg[j] = emit_sel_evict(m, ps, j)
                        # emit recurrence ops as soon as inputs exist; ops on
                        # the post-e4 critical chain stay on DVE (a GpSimd hop
                        # costs ~1.2us), slack ops go to GpSimd
                        if idx == 1:
                            m1 = tmp2("m1")
                            nc.vector.tensor_mul(m1[:], eg[0][:], eg[6][:])
                        elif idx == 2:
                            s01 = tmp2("s01")
                            nc.gpsimd.tensor_add(s01[:], eg[0][:], eg[1][:])
                            t1 = tmp2("t1")
                            nc.gpsimd.tensor_mul(t1[:], eg[1][:], clT[cur][:, hk])
                        elif idx == 3:
                            t2 = tmp2("t2")
                            nc.gpsimd.tensor_mul(t2[:], eg[2][:], cT[cur][:, hk])
                        elif idx == 4:
                            s23 = tmp2("s23")
                            nc.gpsimd.tensor_add(s23[:], eg[2][:], eg[3][:])
                            t3 = tmp2("t3")
                            nc.vector.tensor_mul(t3[:], eg[3][:], crT[cur][:, hk])
                            p12 = tmp2("p12")
                            nc.vector.tensor_add(p12[:], t1[:], t2[:])
                            s0123 = tmp2("s0123")
                            nc.vector.tensor_add(s0123[:], s01[:], s23[:])
                            a012 = tmp2("a012")
                            nc.vector.tensor_add(a012[:], p12[:], m1[:])
                            a0123 = tmp2("a0123")
                            nc.vector.tensor_add(a0123[:], a012[:], t3[:])
                        elif idx == 5:
                            S5 = tmp2("S5", F32)
                            nc.vector.tensor_add(S5[:], s0123[:], eg[4][:])
                            r5 = tmp2("r5", F32)
                            nc.vector.reciprocal_approx_fast(r5[:], S5[:])
                            rm = tmp2("rm")
                            nc.vector.tensor_mul(rm[:], r5[:], mask_sb[:])
                            # acc = a0123 + e4*cg, fused per example
                            acc = tmp2("acc")
                            for e in range(BL):
                                el = slice(e * L, (e + 1) * L)
                                nc.vector.scalar_tensor_tensor(
                                    out=acc[:, el], in0=eg[4][:, el],
                                    scalar=cg_c[:, hk, e : e + 1],
                                    in1=a0123[:, el],
                                    op0=AL.mult, op1=AL.add,
                                )
                            nc.vector.tensor_mul(
                                out=cT[nxt][:, hk], in0=acc[:], in1=rm[:]
                            )
                            tanh_c = tmp2("tanh_c")
                            nc.scalar.activation(
                                tanh_c[:], cT[nxt][:, hk], AF.Tanh
                            )
                            if s < NUM_STEPS - 1:
                                emit_c_shift_copies(nxt, hk)
                    # after the o gate: h_new and its average
                    nc.vector.tensor_mul(
                        out=hT[nxt][:, hk], in0=eg[5][:], in1=tanh_c[:]
                    )
                    if s < NUM_STEPS - 1:
                        # gate GEMM inputs for the next step only
                        emit_shift_copies(nxt, hk)
                    hsum = sm("hsum2")
                    nc.vector.reduce_sum(hsum[:], h3(hT[nxt], hk), axis=AX.X)
                    hav = p_small.tile([128, BL], F8, name="hav", tag="sm")
                    nc.vector.tensor_mul(hav[:], hsum[:], invlen_sb[:])
                    h_avg.append(hav)

                # -- fi GEMM: kc=3 (last-written h chunk) deferred to the end of
                # each accumulation so PE has ready work while h_n[3] finishes
                efims = []
                psfs = []
                for hk in range(HC):
                    psf = psumA.tile([128, N], F32, name="psf", tag="pA")
                    nc.tensor.matmul(
                        psf[:], wfi_sb[:, 0, :, hk * 128 : (hk + 1) * 128],
                        hT[nxt][:, 0:2, :],
                        start=True, stop=False, perf_mode=DR,
                    )
                    nc.tensor.matmul(
                        psf[:],
                        gg_sb[:, 7 * H + hk * 128 : 7 * H + (hk + 1) * 128],
                        sel_sb[:],
                        start=False, stop=False,
                    )
                    psfs.append(psf)
                # keep-warm punctuation: tiny throwaway matmuls that depend on
                # successive points of the hk=3 recurrence chain. They execute
                # spread across the tail wait, so the PE activity monitor never
                # sees a full idle window and the clock stays at 2.4 GHz.
                for dep in (rm[:, :128], acc[:, :128], tanh_c[:, :128]):
                    dmy = psumB.tile([64, 128], F32, name="dmy", tag="pB")
                    nc.tensor.matmul(
                        dmy[:, : dep.free_size()], mask_sb[:, :64], dep,
                        start=True, stop=True,
                    )
                for hk in range(HC):
                    psf = psfs[hk]
                    nc.tensor.matmul(
                        psf[:], wfi_sb[:, 1, :, hk * 128 : (hk + 1) * 128],
                        hT[nxt][:, 2:4, :],
                        start=False, stop=True, perf_mode=DR,
                    )
                    # masked slots already at -1e4 inside psf (sel row BL
                    # trick), so exp gives exact zeros: no mask pass needed
                    efi = tmp2("efi")
                    nc.scalar.activation(
                        efi[:], psf[:], AF.Exp, bias=bfi_sb[:, hk : hk + 1],
                        scale=1.0 / WS,
                    )
                    efims.append(efi)

                # -- fg / og GEMMs (transposed, N=4), in pairs with the
                # h_avg[3] contribution deferred to keep PE fed
                res_sm = {}
                for pair in range(HC):
                    mos = (2 * pair, 2 * pair + 1)
                    psts = []
                    wfp = wfo_sb[:, pair]
                    for wi, mo in enumerate(mos):
                        pst = psumT.tile([128, BL], F32, name="pst_f", tag="pT")
                        for kc in range(2 * HC - 1):
                            rhs = (
                                g_c[:, kc, 0:BL] if kc < HC
                                else h_avg[kc - HC][:]
                            )
                            nc.tensor.matmul(
                                pst[:], wfp[:, wi, kc], rhs,
                                start=(kc == 0), stop=False,
                            )
                        psts.append((pst, wi))
                    for (pst, wi), mo in zip(psts, mos):
                        mm = mo % HC
                        nc.tensor.matmul(
                            pst[:], wfp[:, wi, 2 * HC - 1], h_avg[HC - 1][:],
                            start=False, stop=True,
                        )
                        r_sm = sm("r_sm")
                        if mo < HC:
                            nc.scalar.activation(
                                r_sm[:], pst[:], AF.Exp,
                                bias=bgf_sb[:, mm : mm + 1], scale=1.0 / WS,
                            )
                        else:
                            nc.scalar.activation(
                                r_sm[:], pst[:], AF.Sigmoid,
                                bias=bgo_sb[:, mm : mm + 1], scale=1.0 / WS,
                            )
                        res_sm[mo] = r_sm
                efg = [res_sm[i] for i in range(HC)]
                ogs = [res_sm[HC + i] for i in range(HC)]

                # -- slot softmax + cg/g update (tcg tanh batched at the end:
                # one table load instead of four)
                for hk in range(HC):
                    efim = efims[hk]
                    pw = tmp2("pw")
                    nc.vector.tensor_mul(pw[:], efim[:], cT[nxt][:, hk])
                    s_c = sm("s_c")
                    nc.vector.reduce_sum(s_c[:], v3(pw), axis=AX.X)
                    ssum = sm("ssum")
                    nc.vector.reduce_sum(ssum[:], v3(efim), axis=AX.X)
                    den = sm("den")
                    nc.vector.tensor_add(den[:], efg[hk][:], ssum[:])
                    rden = sm("rden")
                    nc.vector.reciprocal(rden[:], den[:])
                    tnum = sm("tnum")
                    nc.vector.tensor_mul(tnum[:], efg[hk][:], cg_c[:, hk])
                    num = sm("num")
                    nc.vector.tensor_add(num[:], tnum[:], s_c[:])
                    nc.vector.tensor_mul(out=cg_n[:, hk], in0=num[:], in1=rden[:])
                for hk in range(HC):
                    tcg = sm("tcg")
                    nc.scalar.activation(tcg[:], cg_n[:, hk], AF.Tanh)
                    nc.vector.tensor_mul(
                        out=g_n[:, hk, 0:BL], in0=ogs[hk][:], in1=tcg[:]
                    )
                    # keep-warm through the slot-chain tail (f32 lhsT to match
                    # the f32 smalls)
                    dmy = psumB.tile([64, BL], F32, name="dmy2", tag="pB")
                    nc.tensor.matmul(
                        dmy[:], ident_sb[:, :64], tcg[:], start=True, stop=True,
                    )

            # ---------------- epilogue ----------------
            g_fin = gT[NUM_STEPS % 2]
            for pr in range(HC):
                # kc-major within a pair: the kc=0 matmuls start as soon as
                # g_fin chunk 0 lands instead of after the whole tcg loop
                psts = [
                    psumT.tile([128, BL], F32, name="pst_a1", tag="pT")
                    for _ in range(2)
                ]
                for kc in range(HC):
                    for wi in range(2):
                        nc.tensor.matmul(
                            psts[wi][:], w1_sb[:, 2 * pr + wi, kc],
                            g_fin[:, kc, 0:BL],
                            start=(kc == 0), stop=(kc == HC - 1),
                        )
                for wi in range(2):
                    mo = 2 * pr + wi
                    nc.scalar.activation(
                        a1T[:, mo], psts[wi][:], AF.Tanh,
                        bias=b1_sb[:, mo : mo + 1], scale=1.0 / WS,
                    )

            pslg = psumB.tile([BL, DP], F32, name="pslg", tag="pB")
            for kc in range(2 * HC):
                nc.tensor.matmul(
                    pslg[:], a1T[:, kc], w2_sb[:, kc],
                    start=(kc == 0), stop=(kc == 2 * HC - 1),
                )
            lg = p_small.tile([BL, DP], F32, name="lg", tag="lg")
            nc.vector.tensor_add(lg[:], pslg[:], b2_sb[:])
            mx = p_small.tile([BL, 1], F32, name="mx", tag="lg")
            nc.vector.reduce_max(mx[:], lg[:, :DOUT], axis=AX.X)
            tsh = p_small.tile([BL, DOUT], F32, name="tsh", tag="lg")
            nc.vector.tensor_scalar(tsh[:], lg[:, :DOUT], mx[:], None, AL.subtract)
            ex = p_small.tile([BL, DOUT], F32, name="ex", tag="lg")
            ssum = p_small.tile([BL, 1], F32, name="ssum_l", tag="lg")
            nc.scalar.activation(ex[:], tsh[:], AF.Exp, accum_out=ssum[:])
            lse = p_small.tile([BL, 1], F32, name="lse", tag="lg")
            nc.scalar.activation(lse[:], ssum[:], AF.Ln)
            res = p_small.tile([BL, DOUT], F32, name="res", tag="lg")
            nc.vector.tensor_scalar(res[:], tsh[:], lse[:], None, AL.subtract)
            nc.sync.dma_start(out_d.ap(), res[:])

    nc.compile()
    return nc


def prep_in_maps(inputs):
    """Host-side prep: slice per core, pad/retile weights. Returns in_maps."""
    tokens = np.asarray(inputs["tokens"]).astype(np.int32)
    lengths = np.asarray(inputs["lengths"]).astype(np.int32)
    f = lambda k: np.ascontiguousarray(np.asarray(inputs[k], dtype=np.float32))
    embed = f("embed")
    W0, b0 = f("W0"), f("b0")
    Wg, bg = f("Wg"), f("bg")
    Wgf, bgf = f("Wgf"), f("bgf")
    Wfi, bfi = f("Wfi"), f("bfi")
    Wgo, bgo = f("Wgo"), f("bgo")
    W1, b1 = f("W1"), f("b1")
    W2, b2 = f("W2"), f("b2")

    def tile_km(w, kc, mc):
        # [kc*128, mc*128] -> [mc, 128, kc, 128]: piece[m][p,k,c] = w[k*128+p, m*128+c]
        return np.ascontiguousarray(
            w.reshape(kc, 128, mc, 128).transpose(2, 1, 0, 3)
        )

    f8 = ml_dtypes.float8_e4m3fn
    WS, XS = 64.0, 4.0

    def tile_dr(w, scale, mc):
        # [kc*256, mc*128] -> [mc, 128, kc, 2, 128] DoubleRow fp8 pair tiles
        kc = w.shape[0] // 256
        return np.ascontiguousarray(
            (scale * w).reshape(kc, 2, 128, mc, 128).transpose(3, 2, 0, 1, 4)
        ).astype(f8)

    # [m, p, off, jj, i, c]: off 0..2 = hl/h/hr blocks, off 3 = x block
    wg_hhh = (
        (WS * Wg[: 3 * H]).reshape(3, 2, 2, 128, GC, 128).transpose(4, 3, 0, 1, 2, 5)
    )
    EPX = 512
    wg_x_pad = np.zeros((EPX, 7 * H), np.float32)
    wg_x_pad[:E] = (WS / XS) * Wg[3 * H : 3 * H + E]
    wg_xt = wg_x_pad.reshape(2, 2, 128, GC, 128).transpose(3, 2, 0, 1, 4)
    # permute the m axis into kernel emission order (hk-major, J_ORDER minor)
    # and move the partition axis first
    use_order = [j * HC + hk for hk in range(HC) for j in (6, 0, 1, 2, 3, 4, 5)]
    wg_all = np.ascontiguousarray(
        np.concatenate([wg_hhh, wg_xt[:, :, None]], axis=2)[use_order]
        .transpose(1, 0, 2, 3, 4, 5)
    ).astype(f8)
    gcat = WS * np.concatenate([Wg[3 * H + E :], Wfi[:H]], axis=1)  # [512, 4096]
    wg_gcat = np.ascontiguousarray(
        gcat.reshape(2, 2, 128, GGC, 512).transpose(2, 3, 0, 1, 4)
    ).astype(f8)
    wfi_hp = np.ascontiguousarray(
        (WS * Wfi[H:]).reshape(2, 2, 128, H).transpose(2, 0, 1, 3)
    ).astype(f8)
    wgf = tile_km(WS * Wgf, 2 * HC, HC)
    wgo = tile_km(WS * Wgo, 2 * HC, HC)
    wfo = np.ascontiguousarray(
        np.concatenate([wgf, wgo], axis=0)
        .reshape(HC, 2, 128, 2 * HC, 128).transpose(2, 0, 1, 3, 4)
    ).astype(f8)
    w0_pad = np.zeros((EPX, H), np.float32)
    w0_pad[:E] = W0
    w0 = np.ascontiguousarray(np.moveaxis(tile_dr(w0_pad, WS / XS, HC), 1, 0))
    w1 = np.ascontiguousarray(
        np.moveaxis(tile_km(WS * W1, HC, 2 * HC), 1, 0)
    ).astype(f8)
    w2p = np.zeros((2 * H, DP), np.float32)
    w2p[:, :DOUT] = W2
    b2p = np.zeros((DP,), np.float32)
    b2p[:DOUT] = b2
    w2 = np.ascontiguousarray(w2p.reshape(2 * HC, 128, DP).transpose(1, 0, 2))

    def t_bias(b):
        return np.ascontiguousarray(b.reshape(-1, 128).T)

    sel = np.zeros((128, N), np.float32)
    for e in range(BL):
        sel[e, e * L : (e + 1) * L] = 1.0
    ident = np.eye(128, dtype=np.float32)

    bf = ml_dtypes.bfloat16
    shared = dict(
        embed=embed, wg_all=wg_all,
        wg_gcat=wg_gcat, wfi_h=wfi_hp,
        wfo=wfo, w0=w0,
        w1=w1, w2=w2.astype(bf),
        bg_t=t_bias(bg), b0_t=t_bias(b0), bfi_t=t_bias(bfi), bgf_t=t_bias(bgf),
        bgo_t=t_bias(bgo), b1_t=t_bias(b1),
        b2_r=np.ascontiguousarray(np.tile(b2p[None, :], (BL, 1))),
        ident=ident,
    )

    in_maps = []
    for c in range(NCORES):
        sl = slice(c * BL, (c + 1) * BL)
        tok = tokens[sl]                                   # [BL, L]
        lens = np.maximum(lengths[sl].astype(np.float32), 1.0)
        mask = (np.arange(L)[None, :] < lengths[sl][:, None]).astype(np.float32)
        mask_rep = np.ascontiguousarray(
            np.broadcast_to(mask.reshape(1, N), (128, N)).astype(ml_dtypes.bfloat16)
        )
        invlen_rep = np.ascontiguousarray(
            np.broadcast_to((1.0 / lens).reshape(1, BL), (128, BL))
            .astype(ml_dtypes.bfloat16)
        )
        tok_idx = np.ascontiguousarray(tok.T.astype(np.int32))  # [L=128, BL]
        # sel row BL = (1-mask): pairs with gg row BL = -WS*1e4 to mask fi
        sel_m = sel.copy()
        sel_m[BL] = 1.0 - mask.reshape(N)
        m = dict(shared)
        m.update(tok_idx=tok_idx, mask_rep=mask_rep, invlen_rep=invlen_rep,
                 sel_m=sel_m)
        in_maps.append(m)
    return in_maps


_NC_CACHE = {}


def kernel(**inputs) -> np.ndarray:
    in_maps = prep_in_maps(inputs)
    if "nc" not in _NC_CACHE:
        _NC_CACHE["nc"] = build_nc()
    nc = _NC_CACHE["nc"]
    res = run_bass_kernel_spmd(nc, in_maps, core_ids=list(range(NCORES)))
    return np.concatenate([r["out"] for r in res.results], axis=0)


if __name__ == "__main__":
    nc = build_nc()
    print("built ok")

